# revision 31
# baseline (speedup 1.0000x reference)
"""Trainium2 Bass kernel for per-assignment batched linear (moe_routing).

Reference op: out[b, a, o] = sum_i weight[a, o, i] * x[b, a, i] + bias[a, o]
with B=4096, A=32, I=256, O=256, float32.

Sharding: expert-parallel across the 8 NeuronCores — core c owns
assignments [4c, 4c+4).  Each core's shard is x[:, 4c:4c+4, :] plus the
matching weight/bias slices; no cross-core traffic, the gather is a
host-side concatenate.  (Per-core I/O is 2x 8.4 MB + 0.5 MB in bf16 vs
~27 us of TensorE work, so the kernel is HBM-bound; expert-parallel beats
data-parallel because the weights aren't replicated.)

Layout: the contraction dim I must sit on SBUF partitions for the
TensorEngine, so the host pre-transposes x to [A, I, B] and weight to
lhsT tiles [ki, oj].  Per (a, o-chunk) the kernel runs weight-stationary
matmuls out[oj, tok] += wT.T @ xT with a 512-wide moving dim,
accumulating the two I-chunks in fp32 PSUM.  Bias is per-partition in
this orientation and is fused into the PSUM->SBUF eviction
(scalar.activation Identity / vector.tensor_scalar_add with a [128,1]
bias AP).  The output leaves the device as [A_loc, O, B] (bf16) and the
host transposes back and upcasts.

The default implementation ("raw") is a hand-scheduled bacc Block — no
TileContext — measured ~53 us/NEFF vs ~62 us for the Tile version; the
DMA stream is ~99% packed against the ~358 GB/s per-core HBM bound.
Wire dtype is bf16 (fp32 PSUM accumulate, fp32 bias): measured rel err
3.8e-3 vs the fp32 reference.  Set IMPL = "tile-f32r" for a full-fp32
fallback (fp32r matmul, rel err 1.3e-4, ~112 us).

Per-core engine programs (raw impl):
  sync   : w + bias first (they gate the first matmul; this ring starts
           earliest), then x loads — whole per (a, k) for a < 3, column
           QUARTERS for the last assignment so the compute tail gates on
           1/4 of the data rather than all of it
  tensor : 32 groups of 4 matmuls (one 2-bank PSUM tile each, 4 slots);
           the last assignment's two (a, o) pairs are interleaved by
           column block so only the two final-column groups depend on the
           last-arriving x quarter
  scalar : evictions for even column blocks (activation Identity +
           per-partition bias); store triggers — whole per mid-kernel
           (a, o), halves for the tail pair, emitted in readiness order
  vector : evictions for odd column blocks (tensor_scalar_add)
Counting semaphores, all waits are all-done-sound (each wait's sem can
only have been incremented by transfers the waiter needs): sx[a]/sq[q]
(2 DMAs -> 32), sw (w+bias -> 32), smm (+1 per matmul group, PE-sequence
order), sACT/sDVE (+1 per eviction, per-engine order), sst[u] (+16 per
store chunk of out-slot u).
"""

import os

import numpy as np

P = 128
B, A, I, O = 4096, 32, 256, 256
NCORES = 8
A_LOC = A // NCORES  # assignments per core
KC = I // P  # contraction chunks
OC = O // P  # output-row chunks
G = 512  # matmul moving free dim
NG = B // G
E = 2 * G  # eviction width: one 2-bank PSUM tile
M = A_LOC * OC  # (a, o) pairs per core
NGRP = M * (NG // 2)  # matmul groups per core
U = 8  # out_sb slots: one per (a, o) pair, so evictions never carry a
# WAR wait on a previous store's completion (those stalls bite when the
# out stream lags in congested windows); SBUF affords the extra 32 KB/par

IMPL = "raw"  # "raw" (bf16, hand-scheduled) | "tile-bf16" | "tile-f32r"

_NC_CACHE = {}
LAST_RESULT = None  # BassKernelResults of the most recent run (for harnesses)


def _evict_engine_count(m_idx, gg):
    """(is_act, per-engine eviction count) after eviction (m_idx, gg).

    Evictions are interleaved across engines within each (a, o): ScalarE
    takes gg 0 and 2, VectorE takes gg 1 and 3 — so the tail (a, o)'s four
    evictions take two rounds instead of four."""
    is_act = gg % 2 == 0
    return is_act, 2 * m_idx + gg // 2 + 1


def _build_raw():
    from concourse import bacc, mybir

    nc = bacc.Bacc(
        "TRN2", target_bir_lowering=False, debug=False, num_devices=NCORES
    )
    f32 = mybir.dt.float32
    bf16 = mybir.dt.bfloat16
    ident = mybir.ActivationFunctionType.Identity

    x_ext = nc.dram_tensor("x", [A_LOC, KC, P, B], bf16, kind="ExternalInput").ap()
    w_ext = nc.dram_tensor(
        "w", [P, A_LOC * KC * OC * P], bf16, kind="ExternalInput"
    ).ap()
    b_ext = nc.dram_tensor("b", [P, A_LOC * OC], f32, kind="ExternalInput").ap()
    out_ext = nc.dram_tensor(
        "out", [A_LOC, OC, P, B], bf16, kind="ExternalOutput"
    ).ap()

    w_sb = nc.alloc_sbuf_tensor("w_sb", [P, A_LOC * KC * OC * P], bf16).ap()
    b_sb = nc.alloc_sbuf_tensor("b_sb", [P, A_LOC * OC], f32).ap()
    x_sb = [
        nc.alloc_sbuf_tensor(f"x_sb{a}_{k}", [P, B], bf16).ap()
        for a in range(A_LOC)
        for k in range(KC)
    ]
    o_sb = [nc.alloc_sbuf_tensor(f"o_sb{u}", [P, B], bf16).ap() for u in range(U)]
    psum = [nc.alloc_psum_tensor(f"ps{t}", [P, E], f32).ap() for t in range(4)]

    def xi(a, k):
        return a * KC + k

    A_LAST = A_LOC - 1

    # PE group sequence: (m, gg) pairs.  For a < A_LAST, (a,o)-major.  For
    # the last assignment, interleave its two (a, o) pairs by gg so that
    # only the two gg=3 groups depend on the last-arriving x column
    # quarter (the x for a=A_LAST is loaded in column quarters).
    # (Quartering ALL x loads was tried and measured worse — 32 small
    # transfers cost more than the earlier out-stream start buys.)
    group_seq = []
    for m in range(M - OC):
        for gg in range(4):
            group_seq.append((m, gg))
    for gg in range(4):
        for o in range(OC):
            group_seq.append((M - OC + o, gg))
    seq_pos = {mg: i for i, mg in enumerate(group_seq)}

    # Eviction engine split: ACT takes even gg, DVE odd gg; each engine
    # processes its groups in PE-sequence order.
    act_list = [mg for mg in group_seq if mg[1] % 2 == 0]
    dve_list = [mg for mg in group_seq if mg[1] % 2 == 1]
    act_idx = {mg: i + 1 for i, mg in enumerate(act_list)}
    dve_idx = {mg: i + 1 for i, mg in enumerate(dve_list)}

    def _ev_wait(mg):
        """(sem-is-ACT, per-engine count) for 'eviction of group mg done'."""
        if mg[1] % 2 == 0:
            return True, act_idx[mg]
        return False, dve_idx[mg]

    # store chunk plan per (a, o): whole stores mid-kernel (the stream is
    # BW-saturated there), halves for the last pair so the final bytes
    # start moving after two evictions instead of four
    _chunks = {m: (2 if m >= M - OC else 1) for m in range(M)}
    # WAR threshold (in units of 16) for slot m%U at use m: total chunks
    # stored by uses m-U, m-2U, ... of the same slot
    _war_incs = {}
    _cum = [0] * U
    for m in range(M):
        _war_incs[m] = _cum[m % U]
        _cum[m % U] += _chunks[m]

    with (
        nc.Block(no_gpsimd_drain=True) as block,
        nc.semaphore("sx0") as sx0,
        nc.semaphore("sx1") as sx1,
        nc.semaphore("sx2") as sx2,
        nc.semaphore("sq0") as sq0,
        nc.semaphore("sq1") as sq1,
        nc.semaphore("sq2") as sq2,
        nc.semaphore("sq3") as sq3,
        nc.semaphore("sw") as sw,
        nc.semaphore("smm") as smm,
        nc.semaphore("sACT") as sACT,
        nc.semaphore("sDVE") as sDVE,
    ):
        sx = [sx0, sx1, sx2]
        sq = [sq0, sq1, sq2, sq3]
        sst = [nc.alloc_semaphore(f"sst{u}") for u in range(U)]

        @block.sync
        def _(eng):
            # weights/bias first: they gate the first matmul, and this ring
            # starts earlier than scalar's (no ACT_TABLE_LOAD ahead of it)
            eng.dma_start(out=w_sb[:], in_=w_ext[:]).then_inc(sw, 16)
            eng.dma_start(out=b_sb[:], in_=b_ext[:]).then_inc(sw, 16)
            for a in range(A_LAST):
                for k in range(KC):
                    eng.dma_start(out=x_sb[xi(a, k)][:], in_=x_ext[a, k]).then_inc(
                        sx[a], 16
                    )
            # last assignment: column quarters (both k chunks per quarter)
            # so the compute tail gates on 1/4 of the data, not all of it
            Q = B // 4
            for q in range(4):
                for k in range(KC):
                    eng.dma_start(
                        out=x_sb[xi(A_LAST, k)][:, q * Q : (q + 1) * Q],
                        in_=x_ext[A_LAST, k, :, q * Q : (q + 1) * Q],
                    ).then_inc(sq[q], 16)

        @block.tensor
        def _(eng):
            eng.wait_ge(sw, 32)
            cur_a = -1
            cur_q = -1
            for p, (m, gg) in enumerate(group_seq):
                a, o = m // OC, m % OC
                t = p % 4
                if a != cur_a and a < A_LAST:
                    eng.wait_ge(sx[a], 32)
                    cur_a = a
                if a == A_LAST and gg != cur_q:
                    eng.wait_ge(sq[gg], 32)
                    cur_q = gg
                if p >= 4:
                    p_act, cnt = _ev_wait(group_seq[p - 4])
                    eng.wait_ge(sACT if p_act else sDVE, cnt)
                for j in range(2):
                    g = gg * 2 + j
                    for k in range(KC):
                        col = ((a * KC + k) * OC + o) * P
                        mm = eng.matmul(
                            psum[t][:, j * G : (j + 1) * G],
                            w_sb[:, col : col + P],
                            x_sb[xi(a, k)][:, g * G : (g + 1) * G],
                            start=(k == 0),
                            stop=(k == KC - 1),
                        )
                        if j == 1 and k == KC - 1:
                            mm.then_inc(smm)

        def evict_one(eng, m, gg, is_act, war_done):
            """Emit one eviction; WAR-wait on the out slot's previous store
            the first time this engine touches pair m."""
            a, o = m // OC, m % OC
            u = m % U
            bias_ap = b_sb[:, a * OC + o : a * OC + o + 1]
            if m >= U and m not in war_done:
                eng.wait_ge(sst[u], 16 * _war_incs[m])
                war_done.add(m)
            eng.wait_ge(smm, seq_pos[m, gg] + 1)
            dst = o_sb[u][:, gg * E : (gg + 1) * E]
            if is_act:
                eng.activation(dst, psum[seq_pos[m, gg] % 4][:], ident,
                               bias=bias_ap).then_inc(sACT)
            else:
                eng.tensor_scalar_add(
                    dst, psum[seq_pos[m, gg] % 4][:], bias_ap
                ).then_inc(sDVE)

        def emit_store_chunk(eng, m, c):
            """Store chunk c of pair m; waits on the last ACT/DVE evictions
            covering its columns."""
            a, o = m // OC, m % OC
            u = m % U
            nch = _chunks[m]
            W = B // nch
            hi_gg = (c + 1) * 4 // nch - 1
            act_gg = hi_gg if hi_gg % 2 == 0 else hi_gg - 1
            dve_gg = hi_gg if hi_gg % 2 == 1 else hi_gg - 1
            if act_gg >= 0:
                eng.wait_ge(sACT, act_idx[m, act_gg])
            if dve_gg >= 0:
                eng.wait_ge(sDVE, dve_idx[m, dve_gg])
            eng.dma_start(
                out=out_ext[a, o, :, c * W : (c + 1) * W],
                in_=o_sb[u][:, c * W : (c + 1) * W],
            ).then_inc(sst[u], 16)

        @block.scalar
        def _(eng):
            eng.wait_ge(sw, 32)
            war_done = set()
            for i, (m, gg) in enumerate(act_list):
                evict_one(eng, m, gg, True, war_done)
                # mid-kernel pairs: store right after this engine's second
                # eviction of m (waits cover the DVE side)
                if m < M - OC and gg == 2:
                    for c in range(_chunks[m]):
                        emit_store_chunk(eng, m, c)
            # tail pairs: all evictions first, then store chunks in
            # readiness order so no eviction queues behind a blocked store
            for c in range(2):
                for m in range(M - OC, M):
                    emit_store_chunk(eng, m, c)

        @block.vector
        def _(eng):
            eng.wait_ge(sw, 32)
            war_done = set()
            for m, gg in dve_list:
                evict_one(eng, m, gg, False, war_done)

    nc.compile()
    return nc


def _build_tile(io_dtype):
    """Fallback TileContext builder: io_dtype in {"bf16", "f32r"}."""
    import concourse.tile as tile
    from concourse import bacc, mybir

    nc = bacc.Bacc(
        "TRN2", target_bir_lowering=False, debug=False, num_devices=NCORES
    )
    f32 = mybir.dt.float32
    ident = mybir.ActivationFunctionType.Identity
    in_dt = mybir.dt.float32r if io_dtype == "f32r" else mybir.dt.bfloat16
    out_dt = f32 if io_dtype == "f32r" else mybir.dt.bfloat16

    x_ext = nc.dram_tensor("x", [A_LOC, KC, P, B], in_dt, kind="ExternalInput").ap()
    w_ext = nc.dram_tensor(
        "w", [P, A_LOC * KC * OC * P], in_dt, kind="ExternalInput"
    ).ap()
    b_ext = nc.dram_tensor("b", [P, A_LOC * OC], f32, kind="ExternalInput").ap()
    out_ext = nc.dram_tensor(
        "out", [A_LOC, OC, P, B], out_dt, kind="ExternalOutput"
    ).ap()

    xp_bufs = 8 if io_dtype == "bf16" else 4
    op_bufs = 4 if io_dtype == "bf16" else 2

    with tile.TileContext(nc) as tc:
        with (
            tc.tile_pool(name="xp", bufs=xp_bufs) as xp,
            tc.tile_pool(name="wp", bufs=1) as wp,
            tc.tile_pool(name="bp", bufs=1) as bp,
            tc.tile_pool(name="op", bufs=op_bufs) as op_,
            tc.tile_pool(name="pp", bufs=4, space="PSUM") as pp,
        ):
            w_sb = wp.tile([P, A_LOC * KC * OC * P], in_dt)
            nc.scalar.dma_start(out=w_sb[:], in_=w_ext[:])
            b_sb = bp.tile([P, A_LOC * OC], f32)
            nc.scalar.dma_start(out=b_sb[:], in_=b_ext[:])

            for a in range(A_LOC):
                xt = []
                for k in range(KC):
                    t = xp.tile([P, B], in_dt, tag="x")
                    nc.sync.dma_start(out=t[:], in_=x_ext[a, k])
                    xt.append(t)
                for o in range(OC):
                    out_sb = op_.tile([P, B], out_dt, tag="o")
                    bias_ap = b_sb[:, a * OC + o : a * OC + o + 1]
                    for gg in range(NG // 2):
                        ps = pp.tile([P, E], f32)
                        for j in range(2):
                            g = gg * 2 + j
                            for k in range(KC):
                                col = ((a * KC + k) * OC + o) * P
                                nc.tensor.matmul(
                                    ps[:, j * G : (j + 1) * G],
                                    w_sb[:, col : col + P],
                                    xt[k][:, g * G : (g + 1) * G],
                                    start=(k == 0),
                                    stop=(k == KC - 1),
                                )
                        dst = out_sb[:, gg * E : (gg + 1) * E]
                        if gg % 2 == 0:
                            nc.scalar.activation(dst, ps[:], ident, bias=bias_ap)
                        else:
                            nc.vector.tensor_scalar_add(dst, ps[:], bias_ap)
                    nc.scalar.dma_start(out=out_ext[a, o], in_=out_sb[:])

    nc.compile()
    return nc


def _get_nc(impl):
    if impl not in _NC_CACHE:
        if impl == "raw":
            _NC_CACHE[impl] = _build_raw()
        else:
            _NC_CACHE[impl] = _build_tile(impl.split("-")[1])
    return _NC_CACHE[impl]


def kernel(x, weight, bias):
    import ml_dtypes
    from concourse.bass_utils import run_bass_kernel_spmd

    global LAST_RESULT

    # Tracing needs an NTFF hook this container only has when the harness
    # (test.py) installs it; suppress it unless explicitly opted in so a
    # stray BASS_TRACE env can't break the run.
    if os.environ.get("KERNEL_TRACE") != "1":
        os.environ["BASS_NEVER_TRACE"] = "1"

    impl = os.environ.get("KERNEL_IMPL", IMPL)
    np_in = np.float32 if impl == "tile-f32r" else ml_dtypes.bfloat16

    x = np.ascontiguousarray(np.asarray(x), dtype=np.float32)  # [B, A, I]
    weight = np.ascontiguousarray(np.asarray(weight), dtype=np.float32)  # [A, O, I]
    bias = np.ascontiguousarray(np.asarray(bias), dtype=np.float32)  # [A, O]

    # x -> [A, I, B] -> per-core [A_LOC, KC, P, B]
    xT = np.ascontiguousarray(x.transpose(1, 2, 0)).astype(np_in)
    xT = xT.reshape(NCORES, A_LOC, KC, P, B)

    # weight[aG, o*P+oj, k*P+ki] -> w[c][ki, ((a*KC+k)*OC+o)*P + oj]
    w = weight.reshape(NCORES, A_LOC, OC, P, KC, P)  # [c, a, o, oj, k, ki]
    w = np.ascontiguousarray(w.transpose(0, 5, 1, 4, 2, 3)).astype(np_in)
    w = w.reshape(NCORES, P, A_LOC * KC * OC * P)

    # bias[aG, o*P+oj] -> b[c][oj, a*OC+o]
    bb = bias.reshape(NCORES, A_LOC, OC, P)  # [c, a, o, oj]
    bb = np.ascontiguousarray(bb.transpose(0, 3, 1, 2)).reshape(
        NCORES, P, A_LOC * OC
    )

    nc = _get_nc(impl)
    in_maps = [{"x": xT[c], "w": w[c], "b": bb[c]} for c in range(NCORES)]
    res = run_bass_kernel_spmd(nc, in_maps, core_ids=list(range(NCORES)))
    LAST_RESULT = res

    outs = [np.asarray(res.results[c]["out"]) for c in range(NCORES)]
    out = np.concatenate(outs, axis=0)  # [A, OC, P, B]
    out = out.astype(np.float32).reshape(A, O, B).transpose(2, 0, 1)  # [B, A, O]
    return np.ascontiguousarray(out)


if __name__ == "__main__":
    rng = np.random.default_rng(0)
    x = rng.standard_normal((B, A, I), dtype=np.float32)
    weight = rng.standard_normal((A, O, I), dtype=np.float32) / np.sqrt(I)
    bias = rng.standard_normal((A, O), dtype=np.float32)
    out = kernel(x, weight, bias)
    ref = np.einsum("aoi,bai->bao", weight, x) + bias
    err = np.abs(out - ref).max() / np.abs(ref).max()
    print("max-rel-err vs local numpy ref:", err)


# revision 32
# speedup vs baseline: 1.0389x; 1.0389x over previous
"""Trainium2 Bass kernel for per-assignment batched linear (moe_routing).

Reference op: out[b, a, o] = sum_i weight[a, o, i] * x[b, a, i] + bias[a, o]
with B=4096, A=32, I=256, O=256, float32.

Sharding: expert-parallel across the 8 NeuronCores — core c owns
assignments [4c, 4c+4).  Each core's shard is x[:, 4c:4c+4, :] plus the
matching weight/bias slices; no cross-core traffic, the gather is a
host-side concatenate.  (Per-core I/O is 2x 8.4 MB + 0.5 MB in bf16 vs
~27 us of TensorE work, so the kernel is HBM-bound; expert-parallel beats
data-parallel because the weights aren't replicated.)

Layout: the contraction dim I must sit on SBUF partitions for the
TensorEngine, so the host pre-transposes x to [A, I, B] and weight to
lhsT tiles [ki, oj].  Per (a, o-chunk) the kernel runs weight-stationary
matmuls out[oj, tok] += wT.T @ xT with a 512-wide moving dim,
accumulating the two I-chunks in fp32 PSUM.  Bias is per-partition in
this orientation and is fused into the PSUM->SBUF eviction
(scalar.activation Identity / vector.tensor_scalar_add with a [128,1]
bias AP).  The output leaves the device as [A_loc, O, B] (bf16) and the
host transposes back and upcasts.

The default implementation ("raw") is a hand-scheduled bacc Block — no
TileContext — measured ~53 us/NEFF vs ~62 us for the Tile version; the
DMA stream is ~99% packed against the ~358 GB/s per-core HBM bound.
Wire dtype is bf16 (fp32 PSUM accumulate, fp32 bias): measured rel err
3.8e-3 vs the fp32 reference.  Set IMPL = "tile-f32r" for a full-fp32
fallback (fp32r matmul, rel err 1.3e-4, ~112 us).

Per-core engine programs (raw impl):
  sync   : w + bias first (they gate the first matmul; this ring starts
           earliest), then x loads — whole per (a, k) for a < 3, column
           QUARTERS for the last assignment so the compute tail gates on
           1/4 of the data rather than all of it
  tensor : 32 groups of 4 matmuls (one 2-bank PSUM tile each, 4 slots);
           the last assignment's two (a, o) pairs are interleaved by
           column block so only the two final-column groups depend on the
           last-arriving x quarter
  scalar : evictions for even column blocks (activation Identity +
           per-partition bias); store triggers — whole per mid-kernel
           (a, o), halves for the tail pair, emitted in readiness order
  vector : evictions for odd column blocks (tensor_scalar_add)
Counting semaphores, all waits are all-done-sound (each wait's sem can
only have been incremented by transfers the waiter needs): sx[a]/sq[q]
(2 DMAs -> 32), sw (w+bias -> 32), smm (+1 per matmul group, PE-sequence
order), sACT/sDVE (+1 per eviction, per-engine order), sst[u] (+16 per
store chunk of out-slot u).
"""

import os

import numpy as np

P = 128
B, A, I, O = 4096, 32, 256, 256
NCORES = 8
A_LOC = A // NCORES  # assignments per core
KC = I // P  # contraction chunks
OC = O // P  # output-row chunks
G = 512  # matmul moving free dim
NG = B // G
E = 2 * G  # eviction width: one 2-bank PSUM tile
M = A_LOC * OC  # (a, o) pairs per core
NGRP = M * (NG // 2)  # matmul groups per core
U = 8  # out_sb slots: one per (a, o) pair, so evictions never carry a
# WAR wait on a previous store's completion (those stalls bite when the
# out stream lags in congested windows); SBUF affords the extra 32 KB/par

IMPL = "raw"  # "raw" (bf16, hand-scheduled) | "tile-bf16" | "tile-f32r"

_NC_CACHE = {}
LAST_RESULT = None  # BassKernelResults of the most recent run (for harnesses)


def _evict_engine_count(m_idx, gg):
    """(is_act, per-engine eviction count) after eviction (m_idx, gg).

    Evictions are interleaved across engines within each (a, o): ScalarE
    takes gg 0 and 2, VectorE takes gg 1 and 3 — so the tail (a, o)'s four
    evictions take two rounds instead of four."""
    is_act = gg % 2 == 0
    return is_act, 2 * m_idx + gg // 2 + 1


def _build_raw():
    from concourse import bacc, mybir

    nc = bacc.Bacc(
        "TRN2", target_bir_lowering=False, debug=False, num_devices=NCORES
    )
    f32 = mybir.dt.float32
    bf16 = mybir.dt.bfloat16
    ident = mybir.ActivationFunctionType.Identity

    x_ext = nc.dram_tensor("x", [A_LOC, KC, P, B], bf16, kind="ExternalInput").ap()
    w_ext = nc.dram_tensor(
        "w", [P, A_LOC * KC * OC * P], bf16, kind="ExternalInput"
    ).ap()
    b_ext = nc.dram_tensor("b", [P, A_LOC * OC], f32, kind="ExternalInput").ap()
    out_ext = nc.dram_tensor(
        "out", [A_LOC, OC, P, B], bf16, kind="ExternalOutput"
    ).ap()

    w_sb = nc.alloc_sbuf_tensor("w_sb", [P, A_LOC * KC * OC * P], bf16).ap()
    b_sb = nc.alloc_sbuf_tensor("b_sb", [P, A_LOC * OC], f32).ap()
    x_sb = [
        nc.alloc_sbuf_tensor(f"x_sb{a}_{k}", [P, B], bf16).ap()
        for a in range(A_LOC)
        for k in range(KC)
    ]
    o_sb = [nc.alloc_sbuf_tensor(f"o_sb{u}", [P, B], bf16).ap() for u in range(U)]
    psum = [nc.alloc_psum_tensor(f"ps{t}", [P, G], f32).ap() for t in range(8)]

    def xi(a, k):
        return a * KC + k

    A_LAST = A_LOC - 1

    # PE group sequence: (m, gg) pairs.  For a < A_LAST, (a,o)-major.  For
    # the last assignment, interleave its two (a, o) pairs by gg so that
    # only the two gg=3 groups depend on the last-arriving x column
    # quarter (the x for a=A_LAST is loaded in column quarters).
    # (Quartering ALL x loads was tried and measured worse — 32 small
    # transfers cost more than the earlier out-stream start buys.)
    group_seq = []
    for m in range(M - OC):
        for gg in range(8):
            group_seq.append((m, gg))
    for gg in range(8):
        for o in range(OC):
            group_seq.append((M - OC + o, gg))
    seq_pos = {mg: i for i, mg in enumerate(group_seq)}

    # Eviction engine split: ACT takes even gg, DVE odd gg; each engine
    # processes its groups in PE-sequence order.
    act_list = [mg for mg in group_seq if mg[1] % 2 == 0]
    dve_list = [mg for mg in group_seq if mg[1] % 2 == 1]
    act_idx = {mg: i + 1 for i, mg in enumerate(act_list)}
    dve_idx = {mg: i + 1 for i, mg in enumerate(dve_list)}

    def _ev_wait(mg):
        """(sem-is-ACT, per-engine count) for 'eviction of group mg done'."""
        if mg[1] % 2 == 0:
            return True, act_idx[mg]
        return False, dve_idx[mg]

    # store chunk plan per (a, o): whole stores mid-kernel (the stream is
    # BW-saturated there), halves for the last pair so the final bytes
    # start moving after two evictions instead of four
    _chunks = {m: (2 if m >= M - OC else 1) for m in range(M)}
    # WAR threshold (in units of 16) for slot m%U at use m: total chunks
    # stored by uses m-U, m-2U, ... of the same slot
    _war_incs = {}
    _cum = [0] * U
    for m in range(M):
        _war_incs[m] = _cum[m % U]
        _cum[m % U] += _chunks[m]

    with (
        nc.Block(no_gpsimd_drain=True) as block,
        nc.semaphore("sx0") as sx0,
        nc.semaphore("sx1") as sx1,
        nc.semaphore("sx2") as sx2,
        nc.semaphore("sq0") as sq0,
        nc.semaphore("sq1") as sq1,
        nc.semaphore("sq2") as sq2,
        nc.semaphore("sq3") as sq3,
        nc.semaphore("sw") as sw,
        nc.semaphore("smm") as smm,
        nc.semaphore("sACT") as sACT,
        nc.semaphore("sDVE") as sDVE,
    ):
        sx = [sx0, sx1, sx2]
        sq = [sq0, sq1, sq2, sq3]
        sst = [nc.alloc_semaphore(f"sst{u}") for u in range(U)]

        @block.sync
        def _(eng):
            # weights/bias first: they gate the first matmul, and this ring
            # starts earlier than scalar's (no ACT_TABLE_LOAD ahead of it)
            eng.dma_start(out=w_sb[:], in_=w_ext[:]).then_inc(sw, 16)
            eng.dma_start(out=b_sb[:], in_=b_ext[:]).then_inc(sw, 16)
            for a in range(A_LAST):
                for k in range(KC):
                    eng.dma_start(out=x_sb[xi(a, k)][:], in_=x_ext[a, k]).then_inc(
                        sx[a], 16
                    )
            # last assignment: column quarters (both k chunks per quarter)
            # so the compute tail gates on 1/4 of the data, not all of it
            Q = B // 4
            for q in range(4):
                for k in range(KC):
                    eng.dma_start(
                        out=x_sb[xi(A_LAST, k)][:, q * Q : (q + 1) * Q],
                        in_=x_ext[A_LAST, k, :, q * Q : (q + 1) * Q],
                    ).then_inc(sq[q], 16)

        @block.tensor
        def _(eng):
            eng.wait_ge(sw, 32)
            cur_a = -1
            cur_q = -1
            for p, (m, gg) in enumerate(group_seq):
                a, o = m // OC, m % OC
                t = p % 8
                if a != cur_a and a < A_LAST:
                    eng.wait_ge(sx[a], 32)
                    cur_a = a
                if a == A_LAST and gg // 2 != cur_q:
                    eng.wait_ge(sq[gg // 2], 32)
                    cur_q = gg // 2
                if p >= 8:
                    p_act, cnt = _ev_wait(group_seq[p - 8])
                    eng.wait_ge(sACT if p_act else sDVE, cnt)
                for k in range(KC):
                    col = ((a * KC + k) * OC + o) * P
                    mm = eng.matmul(
                        psum[t][:],
                        w_sb[:, col : col + P],
                        x_sb[xi(a, k)][:, gg * G : (gg + 1) * G],
                        start=(k == 0),
                        stop=(k == KC - 1),
                    )
                    if k == KC - 1:
                        mm.then_inc(smm)

        def evict_one(eng, m, gg, is_act, war_done):
            """Emit one eviction; WAR-wait on the out slot's previous store
            the first time this engine touches pair m."""
            a, o = m // OC, m % OC
            u = m % U
            bias_ap = b_sb[:, a * OC + o : a * OC + o + 1]
            if m >= U and m not in war_done:
                eng.wait_ge(sst[u], 16 * _war_incs[m])
                war_done.add(m)
            eng.wait_ge(smm, seq_pos[m, gg] + 1)
            dst = o_sb[u][:, gg * G : (gg + 1) * G]
            if is_act:
                eng.activation(dst, psum[seq_pos[m, gg] % 8][:], ident,
                               bias=bias_ap).then_inc(sACT)
            else:
                eng.tensor_scalar_add(
                    dst, psum[seq_pos[m, gg] % 8][:], bias_ap
                ).then_inc(sDVE)

        def emit_store_chunk(eng, m, c):
            """Store chunk c of pair m; waits on the last ACT/DVE evictions
            covering its columns."""
            a, o = m // OC, m % OC
            u = m % U
            nch = _chunks[m]
            W = B // nch
            hi_gg = (c + 1) * 8 // nch - 1
            act_gg = hi_gg if hi_gg % 2 == 0 else hi_gg - 1
            dve_gg = hi_gg if hi_gg % 2 == 1 else hi_gg - 1
            if act_gg >= 0:
                eng.wait_ge(sACT, act_idx[m, act_gg])
            if dve_gg >= 0:
                eng.wait_ge(sDVE, dve_idx[m, dve_gg])
            eng.dma_start(
                out=out_ext[a, o, :, c * W : (c + 1) * W],
                in_=o_sb[u][:, c * W : (c + 1) * W],
            ).then_inc(sst[u], 16)

        @block.scalar
        def _(eng):
            eng.wait_ge(sw, 32)
            war_done = set()
            for i, (m, gg) in enumerate(act_list):
                evict_one(eng, m, gg, True, war_done)
                # mid-kernel pairs: store right after this engine's second
                # eviction of m (waits cover the DVE side)
                if m < M - OC and gg == 6:
                    for c in range(_chunks[m]):
                        emit_store_chunk(eng, m, c)
            # tail pairs: all evictions first, then store chunks in
            # readiness order so no eviction queues behind a blocked store
            for c in range(2):
                for m in range(M - OC, M):
                    emit_store_chunk(eng, m, c)

        @block.vector
        def _(eng):
            eng.wait_ge(sw, 32)
            war_done = set()
            for m, gg in dve_list:
                evict_one(eng, m, gg, False, war_done)

    nc.compile()
    return nc


def _build_tile(io_dtype):
    """Fallback TileContext builder: io_dtype in {"bf16", "f32r"}."""
    import concourse.tile as tile
    from concourse import bacc, mybir

    nc = bacc.Bacc(
        "TRN2", target_bir_lowering=False, debug=False, num_devices=NCORES
    )
    f32 = mybir.dt.float32
    ident = mybir.ActivationFunctionType.Identity
    in_dt = mybir.dt.float32r if io_dtype == "f32r" else mybir.dt.bfloat16
    out_dt = f32 if io_dtype == "f32r" else mybir.dt.bfloat16

    x_ext = nc.dram_tensor("x", [A_LOC, KC, P, B], in_dt, kind="ExternalInput").ap()
    w_ext = nc.dram_tensor(
        "w", [P, A_LOC * KC * OC * P], in_dt, kind="ExternalInput"
    ).ap()
    b_ext = nc.dram_tensor("b", [P, A_LOC * OC], f32, kind="ExternalInput").ap()
    out_ext = nc.dram_tensor(
        "out", [A_LOC, OC, P, B], out_dt, kind="ExternalOutput"
    ).ap()

    xp_bufs = 8 if io_dtype == "bf16" else 4
    op_bufs = 4 if io_dtype == "bf16" else 2

    with tile.TileContext(nc) as tc:
        with (
            tc.tile_pool(name="xp", bufs=xp_bufs) as xp,
            tc.tile_pool(name="wp", bufs=1) as wp,
            tc.tile_pool(name="bp", bufs=1) as bp,
            tc.tile_pool(name="op", bufs=op_bufs) as op_,
            tc.tile_pool(name="pp", bufs=4, space="PSUM") as pp,
        ):
            w_sb = wp.tile([P, A_LOC * KC * OC * P], in_dt)
            nc.scalar.dma_start(out=w_sb[:], in_=w_ext[:])
            b_sb = bp.tile([P, A_LOC * OC], f32)
            nc.scalar.dma_start(out=b_sb[:], in_=b_ext[:])

            for a in range(A_LOC):
                xt = []
                for k in range(KC):
                    t = xp.tile([P, B], in_dt, tag="x")
                    nc.sync.dma_start(out=t[:], in_=x_ext[a, k])
                    xt.append(t)
                for o in range(OC):
                    out_sb = op_.tile([P, B], out_dt, tag="o")
                    bias_ap = b_sb[:, a * OC + o : a * OC + o + 1]
                    for gg in range(NG // 2):
                        ps = pp.tile([P, E], f32)
                        for j in range(2):
                            g = gg * 2 + j
                            for k in range(KC):
                                col = ((a * KC + k) * OC + o) * P
                                nc.tensor.matmul(
                                    ps[:, j * G : (j + 1) * G],
                                    w_sb[:, col : col + P],
                                    xt[k][:, g * G : (g + 1) * G],
                                    start=(k == 0),
                                    stop=(k == KC - 1),
                                )
                        dst = out_sb[:, gg * E : (gg + 1) * E]
                        if gg % 2 == 0:
                            nc.scalar.activation(dst, ps[:], ident, bias=bias_ap)
                        else:
                            nc.vector.tensor_scalar_add(dst, ps[:], bias_ap)
                    nc.scalar.dma_start(out=out_ext[a, o], in_=out_sb[:])

    nc.compile()
    return nc


def _get_nc(impl):
    if impl not in _NC_CACHE:
        if impl == "raw":
            _NC_CACHE[impl] = _build_raw()
        else:
            _NC_CACHE[impl] = _build_tile(impl.split("-")[1])
    return _NC_CACHE[impl]


def kernel(x, weight, bias):
    import ml_dtypes
    from concourse.bass_utils import run_bass_kernel_spmd

    global LAST_RESULT

    # Tracing needs an NTFF hook this container only has when the harness
    # (test.py) installs it; suppress it unless explicitly opted in so a
    # stray BASS_TRACE env can't break the run.
    if os.environ.get("KERNEL_TRACE") != "1":
        os.environ["BASS_NEVER_TRACE"] = "1"

    impl = os.environ.get("KERNEL_IMPL", IMPL)
    np_in = np.float32 if impl == "tile-f32r" else ml_dtypes.bfloat16

    x = np.ascontiguousarray(np.asarray(x), dtype=np.float32)  # [B, A, I]
    weight = np.ascontiguousarray(np.asarray(weight), dtype=np.float32)  # [A, O, I]
    bias = np.ascontiguousarray(np.asarray(bias), dtype=np.float32)  # [A, O]

    # x -> [A, I, B] -> per-core [A_LOC, KC, P, B]
    xT = np.ascontiguousarray(x.transpose(1, 2, 0)).astype(np_in)
    xT = xT.reshape(NCORES, A_LOC, KC, P, B)

    # weight[aG, o*P+oj, k*P+ki] -> w[c][ki, ((a*KC+k)*OC+o)*P + oj]
    w = weight.reshape(NCORES, A_LOC, OC, P, KC, P)  # [c, a, o, oj, k, ki]
    w = np.ascontiguousarray(w.transpose(0, 5, 1, 4, 2, 3)).astype(np_in)
    w = w.reshape(NCORES, P, A_LOC * KC * OC * P)

    # bias[aG, o*P+oj] -> b[c][oj, a*OC+o]
    bb = bias.reshape(NCORES, A_LOC, OC, P)  # [c, a, o, oj]
    bb = np.ascontiguousarray(bb.transpose(0, 3, 1, 2)).reshape(
        NCORES, P, A_LOC * OC
    )

    nc = _get_nc(impl)
    in_maps = [{"x": xT[c], "w": w[c], "b": bb[c]} for c in range(NCORES)]
    res = run_bass_kernel_spmd(nc, in_maps, core_ids=list(range(NCORES)))
    LAST_RESULT = res

    outs = [np.asarray(res.results[c]["out"]) for c in range(NCORES)]
    out = np.concatenate(outs, axis=0)  # [A, OC, P, B]
    out = out.astype(np.float32).reshape(A, O, B).transpose(2, 0, 1)  # [B, A, O]
    return np.ascontiguousarray(out)


if __name__ == "__main__":
    rng = np.random.default_rng(0)
    x = rng.standard_normal((B, A, I), dtype=np.float32)
    weight = rng.standard_normal((A, O, I), dtype=np.float32) / np.sqrt(I)
    bias = rng.standard_normal((A, O), dtype=np.float32)
    out = kernel(x, weight, bias)
    ref = np.einsum("aoi,bai->bao", weight, x) + bias
    err = np.abs(out - ref).max() / np.abs(ref).max()
    print("max-rel-err vs local numpy ref:", err)
